# revision 50
# baseline (speedup 1.0000x reference)
"""DualAttention Trainium2 kernel (8 NeuronCores, data-parallel over batch).

Math (per batch b, head h, dk=64, S=1024):
  s   = (q @ k^T) / 8                       [S, S]
  E   = exp(s) with strict-causal mask (j < i) applied as -1e30 pre-exp
  Z1  = rowsum(E)
  x   = (E / Z1) * notcm                    in [0, 1]
  E2  = exp(x) ~= 1 + x (Taylor-1, see baseline notes; ~1e-3 Frobenius)
  out = (E2 @ v) / Z2,  Z2 = S + rowsum(x)

With E2 = 1 + x:
  out = bg * r2 + po * r12
where po = E @ (notcm*v), bg = colsum_S(v), S2 = rowsum(E*notcm),
  r12 = 1/(S*Z1 + S2), r2 = Z1 * r12    (since Z2*Z1 = S*Z1 + S2)

Layout: scores are computed TRANSPOSED (s^T[k, q]) so exp's output feeds
the P@V matmuls directly as lhsT. kc-major wide matmuls: for each key
block kc, one (split at psum-bank boundaries) matmul computes
s^T[kc, q >= kc*128] over all later query blocks at once; kc groups
{0},{1,7},{2,6},{3,5},{4} pack into [128,1024] psum tiles so one Exp
activation covers each group (5 ACT instructions per head).

P@V streams a host-packed W = [v1m 64 | v2m 64 | notcm | ones] (130
cols) per (head, kc): one matmul per (qb, kc) accumulates E@v1m, E@v2m,
S2 and Z1 together. po lives in 1-bank psum tiles of 3 qb x 130.

Final: DVE scales po*r12 into SBUF; Pool adds bg*r2 and writes bf16
into per-half big tiles ([p, c, h, dk] layout); 4 fat output DMAs; host
unshuffles and upcasts to f32.
"""

import numpy as np

import concourse.bass as bass
import concourse.mybir as mybir
from concourse.tile import TileContext
from concourse.alu_op_type import AluOpType

F32 = mybir.dt.float32
BF16 = mybir.dt.bfloat16
I32 = mybir.dt.int32

B, S, D = 8, 1024, 1024
H, DK = 16, 64
NCORES = 8
P = 128          # partition block
NQB = S // P     # 8 query/key blocks
MASKADD = -1e30
WC = 130         # W columns per chunk: v1 64 | v2 64 | notcm | ones

# kc grouping into [128, 1024] psum tiles: widths (8-kc)*128
KC_GROUPS = [(0,), (1, 7), (2, 6), (3, 5), (4,)]
# column base of each kc's region inside pp (group-major)
KC_BASE = {}
_off = 0
for _g in KC_GROUPS:
    for _kc in _g:
        KC_BASE[_kc] = _off
        _off += (NQB - _kc) * P
TOTW = _off      # 4608
# base offset of each group inside its own psum tile
G_BASE = []
for _g in KC_GROUPS:
    _b, _bs = 0, []
    for _kc in _g:
        _bs.append(_b)
        _b += (NQB - _kc) * P
    G_BASE.append((_bs, _b))  # (per-kc offset, total width)

TRIPLETS = [(0, 1, 2), (3, 4, 5), (6, 7)]

# groups whose exp runs on DVE via Schraudolph bit-trick (real-HW ACT is
# ~1.13 ns/col while DVE does ~0.33 ns/col/op; splitting balances them).
# exp(x) ~= bitcast_f32(int32(A*x + B)); element noise ~3% is cancelled
# by the softmax normalization (verified end-to-end ~1e-3).
DVE_EXP_GROUPS = (0, 1)
SCHR_A = float(2 ** 23 / np.log(2.0)) * 0.125  # folds the 1/8 score scale
SCHR_B = float(127.0 * 2 ** 23 - 60801.48)


def build_nc(reps=1, stages=5):
    # reps>1 repeats the main loop inside one NEFF — used only by the
    # timing harness (marginal wall time per rep == device main-loop
    # time, with the axon dispatch offset cancelled). stages truncates
    # the pipeline for timing attribution: 1=scores 2=+exp 3=+PV/finish
    # 4=+scales 5=+out-DMA (full).
    from concourse.bacc import Bacc

    nc = Bacc()
    # host passes q/k pre-transposed [D, S] bf16; W packed per
    # (pair, head, kc): [128, H*NQB*WC]; bg = per-head colsum rows
    # [128(bcast), H*128] bf16; outputs in big-tile layout
    # [128, 2 halves * NQB * 8 * DK] bf16, host unshuffles.
    qt_d = nc.declare_dram_parameter("qT", [D, S], BF16, isOutput=False)
    kt_d = nc.declare_dram_parameter("kT", [D, S], BF16, isOutput=False)
    w_d = nc.declare_dram_parameter("W", [P, H * NQB * WC], BF16, isOutput=False)
    # bg in row-select form: [3 rows, H * 3*WC]; block i is bg_h (padded
    # to WC) on row i, zero elsewhere, so one matmul per triplet
    # accumulates Z1[q]*bg[d] into the whole po tile
    bg_d = nc.declare_dram_parameter("bg", [3, H * 3 * WC], BF16,
                                     isOutput=False)
    # both outputs interleaved in per-pair big-tile layout
    # [p, pair, c, hh, v1 64 | v2 64]; host splits + unshuffles
    o12_d = nc.declare_dram_parameter("out12", [P, 2 * NQB * S], BF16, isOutput=True)

    from contextlib import ExitStack

    with TileContext(nc) as tc, ExitStack() as ctx:
        const = ctx.enter_context(tc.tile_pool(name="const", bufs=1))
        qkpool = ctx.enter_context(tc.tile_pool(name="qk", bufs=2))
        wpool = ctx.enter_context(tc.tile_pool(name="wp", bufs=2))
        pppool = ctx.enter_context(tc.tile_pool(name="pp", bufs=2))
        smol = ctx.enter_context(tc.tile_pool(name="sm", bufs=4))
        ipool = ctx.enter_context(tc.tile_pool(name="ip", bufs=2))
        bigp = ctx.enter_context(tc.tile_pool(name="big", bufs=2))
        # PSUM budget (8 banks): ps 2x2 + po 3x1 + zT 1
        ps_pool = ctx.enter_context(tc.tile_pool(name="ps", bufs=2, space="PSUM"))
        po_pool = ctx.enter_context(tc.tile_pool(name="po", bufs=3, space="PSUM"))
        zt_pool = ctx.enter_context(tc.tile_pool(name="zt", bufs=1, space="PSUM"))

        # ---------------- constants ----------------
        # touch Exp immediately so the ~2.7us ACT table load overlaps the
        # first input DMAs instead of stalling the first exp
        warm = const.tile([1, 1], F32, tag="warm")
        nc.gpsimd.memset(warm[:], 0.0)
        nc.scalar.activation(out=warm[:], in_=warm[:],
                             func=mybir.ActivationFunctionType.Exp)

        ident = const.tile([P, P], BF16, tag="ident")
        nc.gpsimd.memset(ident[:], 0.0)
        nc.gpsimd.affine_select(
            out=ident[:], in_=ident[:], compare_op=AluOpType.not_equal,
            fill=1.0, base=0, pattern=[[-1, P]], channel_multiplier=1)

        # ------------- main loop: 16 heads, pipelined ------
        state = {}

        def stage_load(hp):
            if hp >= NQB or ("pair", hp) in state:
                return
            dsl = slice(hp * P, (hp + 1) * P)
            qT2 = qkpool.tile([P, S], BF16, tag="qT2")
            kT2 = qkpool.tile([P, S], BF16, tag="kT2")
            wpr = wpool.tile([P, 2 * NQB * WC], BF16, tag="wpr")
            if hp == 0:
                # split so head 0's first score matmuls (kc=0 lhsT block,
                # head-0 q rows) unblock as early as possible
                nc.sync.dma_start(out=kT2[:, 0:P], in_=kt_d[dsl, 0:P])
                nc.sync.dma_start(out=qT2[0:DK, :], in_=qt_d[0:DK, :])
                nc.sync.dma_start(out=kT2[:, P:], in_=kt_d[dsl, P:])
                nc.sync.dma_start(out=qT2[DK:P, :], in_=qt_d[DK:P, :])
            else:
                nc.sync.dma_start(out=qT2[:], in_=qt_d[dsl, :])
                nc.sync.dma_start(out=kT2[:], in_=kt_d[dsl, :])
            nc.sync.dma_start(
                out=wpr[:],
                in_=w_d[:, 2 * hp * NQB * WC:(2 * hp + 2) * NQB * WC])
            state[("pair", hp)] = (qT2, kT2, wpr)

        bg_all = const.tile([3, H * 3 * WC], BF16, tag="bgall")

        def stage_a(h, groups):
            """Transposed scores + exp for the given kc groups of head h."""
            hp, hl = divmod(h, 2)
            stage_load(hp)
            if stages < 1:
                return
            qT2, kT2, _ = state[("pair", hp)]
            pb = hl * DK
            if h not in state:
                pp = pppool.tile([P, TOTW], BF16, tag="pp")
                state[h] = dict(pb=pb, pp=pp)
            pp = state[h]["pp"]
            for gi in groups:
                kcs = KC_GROUPS[gi]
                offs, width = G_BASE[gi]
                ps = ps_pool.tile([P, 2 * P * 4], F32, tag="ps")  # 1024 f32
                for kc, off in zip(kcs, offs):
                    w = (NQB - kc) * P
                    # split at psum bank boundaries (mult of 512 in-tile)
                    segs = []
                    c = 0
                    while c < w:
                        nxt = min(w, ((off + c) // 512 + 1) * 512 - off)
                        segs.append((c, nxt))
                        c = nxt
                    for c0, c1 in segs:
                        nc.tensor.matmul(
                            ps[:, off + c0: off + c1],
                            kT2[pb: pb + DK, kc * P: (kc + 1) * P],
                            qT2[pb: pb + DK, kc * P + c0: kc * P + c1],
                            start=True, stop=True)
                if stages < 2:
                    continue
                gb = KC_BASE[kcs[0]]
                if gi in DVE_EXP_GROUPS:
                    yi = ipool.tile([P, 2 * P * 4], I32, tag="yi")
                    nc.vector.tensor_scalar(
                        out=yi[:, 0: width], in0=ps[:, 0: width],
                        scalar1=SCHR_A, scalar2=SCHR_B,
                        op0=AluOpType.mult, op1=AluOpType.add)
                    nc.vector.tensor_copy(
                        pp[:, gb: gb + width], yi[:, 0: width].bitcast(F32))
                else:
                    nc.scalar.activation(
                        out=pp[:, gb: gb + width], in_=ps[:, 0: width],
                        func=mybir.ActivationFunctionType.Exp, scale=0.125)
                for kc in kcs:
                    # strict-causal zero of the diagonal chunk (keep k < q)
                    dg = pp[:, KC_BASE[kc]: KC_BASE[kc] + P]
                    nc.gpsimd.affine_select(
                        out=dg, in_=dg, compare_op=AluOpType.is_ge,
                        fill=0.0, base=-1, pattern=[[1, P]],
                        channel_multiplier=-1)

        def stage_bc_t(h, t, big12):
            """P@V for triplet t; on the last triplet: z-math, bg
            outer-product accumulation, and final scales for head h."""
            if stages < 3:
                return
            st = state[h]
            hp, hl = divmod(h, 2)
            _, _, wpr = state[("pair", hp)]
            pb, pp = st["pb"], st["pp"]
            wbase = hl * NQB * WC
            hh = hl
            if "r12" not in st:
                st["r12"] = smol.tile([P, NQB], F32, tag="r12", name="r12")
                st["zs"] = smol.tile([P, NQB * 2], F32, tag="zs", name="zs")
                st["z1c"] = smol.tile([P, NQB], BF16, tag="z1c", name="z1c")
                st["z1r"] = smol.tile([3, 3 * P], BF16, tag="z1r", name="z1r")
            r12, zs = st["r12"], st["zs"]
            zsv = zs.rearrange("p (q c) -> p q c", c=2)
            qbs = TRIPLETS[t]
            nt = len(qbs)
            po = po_pool.tile([P, 3 * WC], F32, tag="po")
            for i, qb in enumerate(qbs):
                for kc in range(qb + 1):
                    lhs = pp[:, KC_BASE[kc] + (qb - kc) * P:
                             KC_BASE[kc] + (qb - kc + 1) * P]
                    nc.tensor.matmul(
                        po[:, i * WC: (i + 1) * WC],
                        lhs, wpr[:, wbase + kc * WC: wbase + (kc + 1) * WC],
                        start=(kc == 0), stop=False)
            t0 = qbs[0]
            pov = po.rearrange("p (q c) -> p q c", c=WC)
            # (S2, Z1) columns out of psum before the bg accumulation
            # lands on them (vector ops allow only one PSUM operand)
            nc.vector.tensor_copy(
                zsv[:, t0: t0 + nt, :], pov[:, 0:nt, P: P + 2])
            z1c, z1r = st["z1c"], st["z1r"]
            # this triplet's Z1 as rows (bf16 copy, PE transpose, back to
            # SBUF) — emitted before the r-math so the transpose chain and
            # the reciprocal chain run in parallel
            nc.vector.tensor_copy(
                z1c[:, t0: t0 + nt], zsv[:, t0: t0 + nt, 1])
            z1t = zt_pool.tile([3, P], BF16, tag="z1t")
            nc.tensor.transpose(z1t[0:nt, :], z1c[:, t0: t0 + nt], ident[:])
            z1rt = z1r[0:nt, t * P: (t + 1) * P]
            nc.vector.tensor_copy(z1rt, z1t[0:nt, :])
            # r12 = 1/(S*Z1 + S2) for this triplet's qbs
            nc.vector.scalar_tensor_tensor(
                out=r12[:, t0: t0 + nt], in0=zsv[:, t0: t0 + nt, 1],
                scalar=float(S), in1=zsv[:, t0: t0 + nt, 0],
                op0=AluOpType.mult, op1=AluOpType.add)
            nc.vector.reciprocal(r12[:, t0: t0 + nt], r12[:, t0: t0 + nt])
            if t0 == 0:
                # global row 0: empty window, forced zero output
                nc.gpsimd.memset(r12[0:1, 0:1], 0.0)
            # po += Z1[q] * bg[d]  (row-select outer product, one matmul,
            # closes the psum accumulation over the whole triplet)
            base = h * 3 * WC
            nc.tensor.matmul(
                po[:, 0: nt * WC], z1rt,
                bg_all[0:nt, base: base + nt * WC],
                start=False, stop=True)
            # out = po * r12, straight into the big tile (bf16)
            if stages >= 4:
                for i, qb in enumerate(qbs):
                    nc.vector.tensor_scalar_mul(
                        big12[:, qb * 2 * P + hh * P:
                              qb * 2 * P + (hh + 1) * P],
                        pov[:, i, 0:P], r12[:, qb: qb + 1])
            if t == len(TRIPLETS) - 1:
                state.pop(h)

        for _rep in range(reps):
            state.clear()
            # first pair's q/k ahead of bg so scores start early
            stage_load(0)
            if _rep == 0:
                nc.sync.dma_start(out=bg_all[:], in_=bg_d[:])
            bigs = None
            def bc(h, t):
                b12 = state[("bigs", h // 2)]
                stage_bc_t(h, t, b12)
                if stages < 5:
                    return
                if t == len(TRIPLETS) - 1 and h % 2 == 1:
                    cols = NQB * 2 * P
                    base = (h // 2) * cols
                    nc.sync.dma_start(
                        out=o12_d[:, base: base + cols], in_=b12[:])
                    state.pop(("bigs", h // 2))

            for it in range(H + 1):
                if it < H:
                    if it % 2 == 0:
                        state[("bigs", it // 2)] = bigp.tile(
                            [P, NQB * 2 * P], BF16, tag="big12",
                            name=f"big12_{it}")
                    stage_a(it, (0, 1, 2))
                    if it % 2 == 0:
                        stage_load(it // 2 + 1)  # prefetch next pair
                    stage_a(it, (3,))
                    if it == H - 1:
                        # last head: overlap its own P@V with its last exps
                        bc(it - 1, 0)
                        bc(it - 1, 1)
                        bc(it - 1, 2)
                        stage_a(it, (4,))
                        bc(it, 0)
                    else:
                        stage_a(it, (4,))
                        if it >= 1:
                            for t in range(len(TRIPLETS)):
                                bc(it - 1, t)
                else:
                    bc(it - 1, 1)
                    bc(it - 1, 2)
    nc.compile()
    return nc


_NC_CACHE = None


def _get_nc():
    global _NC_CACHE
    if _NC_CACHE is None:
        _NC_CACHE = build_nc()
    return _NC_CACHE


def prep_inputs(q, k, v1, v2, counter_attention_mask):
    """Host-side shard prep: transpose q/k per batch, pack W, bg; bf16."""
    import ml_dtypes

    bf = ml_dtypes.bfloat16
    q = np.asarray(q, dtype=np.float32)
    k = np.asarray(k, dtype=np.float32)
    v1 = np.asarray(v1, dtype=np.float32)
    v2 = np.asarray(v2, dtype=np.float32)
    cm = np.asarray(counter_attention_mask)
    notcm = (cm == 0).astype(np.float32)  # [B, S]
    v1m = v1 * notcm[:, :, None]
    v2m = v2 * notcm[:, :, None]
    # W[b, p, h, c, :] = [v1m[c*128+p, h*64:+64] | v2m[...] | notcm | 1]
    W = np.empty((B, P, H, NQB, WC), dtype=np.float32)
    v1r = v1m.reshape(B, NQB, P, H, DK).transpose(0, 2, 3, 1, 4)  # b p h c dk
    v2r = v2m.reshape(B, NQB, P, H, DK).transpose(0, 2, 3, 1, 4)
    W[..., 0:DK] = v1r
    W[..., DK:2 * DK] = v2r
    W[..., 2 * DK] = notcm.reshape(B, NQB, P).transpose(0, 2, 1)[:, :, None, :]
    W[..., 2 * DK + 1] = 1.0
    W = W.reshape(B, P, H * NQB * WC).astype(bf)
    # bg in row-select form [8, H, 3, 3, WC]: triplet t block i carries
    # [colsum(v1h) | colsum(v2h) | 0 0] on row TRIPLETS[t][i] only
    bg1 = v1.sum(axis=1, dtype=np.float64).reshape(B, H, DK)
    bg2 = v2.sum(axis=1, dtype=np.float64).reshape(B, H, DK)
    bgrow = np.zeros((B, H, WC), dtype=np.float32)
    bgrow[..., 0:DK] = bg1
    bgrow[..., DK:2 * DK] = bg2
    bg = np.zeros((B, 3, H, 3, WC), dtype=np.float32)
    for i in range(3):
        bg[:, i, :, i, :] = bgrow
    bg = bg.reshape(B, 3, H * 3 * WC).astype(bf)
    return [
        {"qT": np.ascontiguousarray(q[b].astype(bf).T),
         "kT": np.ascontiguousarray(k[b].astype(bf).T),
         "W": np.ascontiguousarray(W[b]),
         "bg": np.ascontiguousarray(bg[b])}
        for b in range(NCORES)
    ]


def _unshuffle(res):
    # out12 [P, pair, c, hh, vsel, dk] -> two [S, D] f32 arrays
    a = res["out12"].reshape(P, 8, NQB, 2, 2, DK).astype(np.float32)
    a = a.transpose(4, 2, 0, 1, 3, 5).reshape(2, S, D)
    return a[0], a[1]


def kernel(q, k, v1, v2, counter_attention_mask):
    from concourse.bass_utils import run_bass_kernel_spmd

    in_maps = prep_inputs(q, k, v1, v2, counter_attention_mask)
    nc = _get_nc()
    res = run_bass_kernel_spmd(nc, in_maps, list(range(NCORES))).results
    outs = [_unshuffle(res[b]) for b in range(NCORES)]
    out1 = np.stack([o[0] for o in outs])
    out2 = np.stack([o[1] for o in outs])
    return out1, out2
